# revision 26
# baseline (speedup 1.0000x reference)
"""Causal self-attention on 8 Trainium2 NeuronCores.

Tensor-parallel by heads: each core owns 2 of the 16 heads end-to-end
(QKV projection -> causal attention -> row-sharded output projection),
and the 8 partial projection outputs are summed on the host.

Schedule (v2, ~177us vs 185us for the first working version):
  - exp is split 3:2 between the Scalar ACT engine (spline Exp) and a
    custom DVE op EXP_SQ16_ANT (exp(s*x) = ((s'^2/2 x + s') x + 1)^16,
    s' = s/16; exactly 8 ALU stages; rel err <6e-3 over the observed
    score range, registered additively into concourse.dve_ops at
    import) so the attention phase is no longer ACT-exp-bound.
  - normalize is split: denominator-row copies + DRAM-bounce broadcast
    DMAs are emitted at each q-tile's last attnV, but the dependent
    reciprocal + multiplies are emitted ~4 groups later so the DMA
    round-trip latency never head-blocks the DVE FIFO.
  - all DRAM tensors use tile-matched layouts ([g,p,ko,m] x, [p,ko,n]
    w, [g,p,s,c] out) so every DMA is 128 contiguous 8KB spans --
    descriptor-count, issue cost and per-ring throughput all improve
    ~8x over row-major layouts.
  - x tiles ride the sync queue in consumption order (per-queue FIFO
    serializes transfers: tile g always lands before g+1); outputs are
    batched 4 token-blocks per DMA, issued on sync for b0 (idle during
    b1's QKV) and gpsimd for b1 (clean at the kernel tail); the
    denominator bounce uses the opposite queue per batch.
  - proj(b0) is interleaved into b1's QKV chains (evac slack is
    plentiful there); proj(b1) runs at the tail with whole-sm [128,
    1024] evacuations on the ps ring so two engines keep up with
    back-to-back proj matmuls.
  - psum evacuations alternate ACT/DVE (GpSimd cannot access PSUM);
    q/k bias adds keep their scalar operand pre-copied on the reading
    engine (TensorScalarPtr has a single wait slot).
"""

import os
import numpy as np
from contextlib import ExitStack

import concourse.bass as bass
import concourse.mybir as mybir
import concourse.tile as tile
from concourse import bacc

B, T, C, H, D = 2, 2048, 1024, 16, 64
NCORES = 8
HPC = H // NCORES          # heads per core = 2
BT = B * T                 # 4096 tokens
P = 128
KO = C // P                # 8 contraction chunks of 128
MT = 512                   # qkv m-tile (tokens)
NMT_B = T // MT            # 4 m-tiles per batch
QTW = 512                  # q tile width
NQT = T // QTW             # 4
NKB = T // P               # 16 k-blocks per batch
SCALE = 1.0 / np.sqrt(D)   # 0.125
F32 = mybir.dt.float32
BF16 = mybir.dt.bfloat16
MMDT = BF16

LAST_RESULT = None  # BassKernelResults of the most recent run (for profiling)

# ---------------------------------------------------------------------------
# Custom DVE op: exp(SCALE*x) ~= ((x*c0 + c1)*x + 1)^16 with c1 = SCALE/16,
# c0 = c1^2/2.  Exactly 8 ALU stages (mult, add, mult, add, 4x square).
# Registered into concourse.dve_ops' tables at import (additive only).
# ---------------------------------------------------------------------------
_EXP_C1 = float(SCALE / 16.0)
_EXP_C0 = float(_EXP_C1 * _EXP_C1 / 2.0)


def _exp_sq16_ref(in0, in1, s0, s1, imm2):
    x = in0.astype(np.float32)
    q = (x * np.float32(s0) + np.float32(s1)) * x + np.float32(1.0)
    for _ in range(4):
        q = (q * q).astype(np.float32)
    return q


def _register_exp_op():
    import concourse.dve_ops as dve_ops
    import concourse.dve_spec as dve_spec
    from concourse.dve_spec import Spec, Src0, C0, C1, One, sq
    from concourse.dve_uop import DveOpSpec

    name = "EXP_SQ16_ANT"
    for op in dve_ops.OPS:
        if op.name == name:
            return op
    spec = Spec(
        body=sq(sq(sq(sq((Src0 * C0 + C1) * Src0 + One)))),
        reference=_exp_sq16_ref,
    )
    row = dve_ops._CUSTOM_DVE_ROW_BASE + len(dve_ops.OPS)
    assert row < 0x20, "no free custom-DVE opcode row"
    shas = {}
    for ver in ("v3", "v4"):
        try:
            uops = dve_spec.lower(spec, ver=ver)
            shas[ver] = DveOpSpec(
                name=name, opcode=row, uops=uops, rd1_en=False
            ).sha(ver)
        except Exception:
            pass
    op = dve_ops.DveOp(name, spec, subdim=False, uops_sha=shas)
    dve_ops.OPS.append(op)
    dve_ops.CUSTOM_DVE_SPECS[name] = spec
    dve_ops._SUB_OPCODE_FOR_NAME[name] = row
    return op


try:
    EXP_OP = _register_exp_op()
except Exception:
    EXP_OP = None

# which groups' exp runs on the DVE custom op (the rest on ACT).  2-of-5
# keeps DVE below the PE's per-group budget alongside its normalize work.
_DVE_PAT = os.environ.get("KERNEL_DVE_EXP", "13")


def build_nc():
    nc = bacc.Bacc(target_bir_lowering=False)

    NG = B * NMT_B  # 8 tile-groups of 512 tokens
    xT_d = nc.dram_tensor("xT", [NG, P, KO, MT], MMDT, kind="ExternalInput")
    w_d = nc.dram_tensor("w", [P, 3, KO, HPC * D], MMDT, kind="ExternalInput")
    bqk_d = nc.dram_tensor("bqk", [P, 2], F32, kind="ExternalInput")
    wp_d = nc.dram_tensor("wp", [P, C], MMDT, kind="ExternalInput")
    id_d = nc.dram_tensor("ident", [P, P], F32, kind="ExternalInput")
    out_d = nc.dram_tensor("out", [NG, P, 4, C], BF16, kind="ExternalOutput")

    xT4 = xT_d.ap()      # [8, 128, 8, 512] tile-matched
    w4 = w_d.ap()        # [128, 3, 8, 128] nch-major
    out4 = out_d.ap()    # [8, 128, 4, 1024] tile-matched

    Exp = mybir.ActivationFunctionType.Exp
    Copy = mybir.ActivationFunctionType.Copy
    mult = mybir.AluOpType.mult
    add = mybir.AluOpType.add

    with tile.TileContext(nc) as tc, ExitStack() as ctx:
        const = ctx.enter_context(tc.tile_pool(name="const", bufs=1))
        big = ctx.enter_context(tc.tile_pool(name="big", bufs=1))
        xpool = ctx.enter_context(tc.tile_pool(name="xpool", bufs=8))
        epool = ctx.enter_context(tc.tile_pool(name="epool", bufs=8))
        spool = ctx.enter_context(tc.tile_pool(name="spool", bufs=2))
        opool = ctx.enter_context(tc.tile_pool(name="opool", bufs=4))
        ypool = ctx.enter_context(tc.tile_pool(name="ypool", bufs=8))
        dpool = ctx.enter_context(tc.tile_pool(name="dpool", bufs=2, space="DRAM"))
        psum = ctx.enter_context(tc.tile_pool(name="psum", bufs=1, space="PSUM"))

        # ---------------- input DMAs ----------------
        # sync queue: x g0 per-k-chunk (interleaved with w on the scalar
        # queue, so the k-th QKV matmul starts when its own slices land).
        # vector queue: wp/ident/bqk consts.  remaining x m-tiles are
        # prefetched whole on rotating engine queues.
        w_sb = const.tile([P, 3, KO, HPC * D], MMDT)
        wp_sb = const.tile([P, C], MMDT)
        id_sb = const.tile([P, P], F32)
        bqk_sb0 = const.tile([P, 2], F32)
        if os.environ.get("KERNEL_PB2", "0") == "1":
            from concourse import library_config
            nc.gpsimd.load_library(library_config.proxy)

        # few big DMA issues (descriptor generation costs ~0.7us per issue
        # regardless of size).  w first on scalar; all x tiles on sync in
        # consumption order (per-queue FIFO serializes transfers, so g_k
        # lands before g_{k+1}).
        for nch in range(3):
            nc.scalar.dma_start(out=w_sb[:, nch], in_=w4[:, nch])
        nc.gpsimd.dma_start(out=wp_sb[:, :], in_=wp_d.ap()[:, :])
        nc.gpsimd.dma_start(out=id_sb[:], in_=id_d.ap())
        nc.gpsimd.dma_start(out=bqk_sb0[:], in_=bqk_d.ap())

        xms = []
        for g in range(B * NMT_B):
            xms.append(xpool.tile([P, KO, MT], MMDT, tag="xm", name=f"xm{g}"))
        for g in range(B * NMT_B):
            nc.sync.dma_start(out=xms[g][:, :, :], in_=xT4[g])

        # Pre-consume DMA semaphores on the engines that will read these
        # tiles (single-wait-slot encodings can't wait (engine, DMA)).
        bqk_q = const.tile([P, 2], F32)     # read by vector (q/k bias)
        nc.vector.tensor_copy(out=bqk_q[:], in_=bqk_sb0[:])
        idb_sb = const.tile([P, P], MMDT)   # read by PE transposes
        nc.scalar.copy(out=idb_sb[:], in_=id_sb[:])

        QT_sb = big.tile([P, B, T], MMDT)   # rows: [qA feats | qB feats]
        KT_sb = big.tile([P, B, T], MMDT)
        VT_sb = big.tile([P, B, T], MMDT)
        # merged V tile: cols 0:64 vA, 64 onesA | 65.. : VB block
        # (65+32 onesB, 65+64..65+128 vB)
        VAB = big.tile([P, B, NKB, 65 + P], MMDT)
        nc.gpsimd.memset(VAB[:], 0.0)
        nc.vector.memset(VAB[:, :, :, 64:65], 1.0)
        nc.vector.memset(VAB[:, :, :, 65 + 32:65 + 33], 1.0)

        warmed = [False]

        def emit_warmup():
            # PE pre-consume of wp/id DMA sems (results unused), emitted
            # after the first QKV chain so it doesn't gate the PE start.
            warmed[0] = True
            pid = psum.tile([P, P], F32, tag="py", bufs=4)
            nc.tensor.transpose(pid[:], id_sb[:], id_sb[:])
            pwp = psum.tile([P, QTW], F32, tag="py", bufs=4)
            nc.tensor.matmul(pwp[:, 0:P], wp_sb[:, 0:P], wp_sb[:, 0:P],
                             start=True, stop=True)
            nc.tensor.matmul(pwp[:, 0:P], wp_sb[:, QTW:QTW + P],
                             wp_sb[:, QTW:QTW + P], start=True, stop=True)

        yts = {}
        pys = {}
        evac_rr = [0]

        def emit_qkv(b, proj_b=None):
            for mt in range(NMT_B):
                g = b * NMT_B + mt
                xm = xms[g]
                if proj_b is not None:
                    for sm in range(mt * 4, mt * 4 + 4):
                        emit_proj_sm(proj_b, sm, whole=False)
                for nch in range(3):
                    pq = psum.tile([P, MT], F32, tag="py", bufs=4)
                    for k in range(KO):
                        nc.tensor.matmul(
                            pq[:],
                            (w_sb[:, nch, k, :]),
                            (xm[:, k, :]),
                            start=(k == 0),
                            stop=(k == KO - 1),
                        )
                    dst = (QT_sb, KT_sb, VT_sb)[nch]
                    dslice = dst[:, b, mt * MT:(mt + 1) * MT]
                    if nch == 0:
                        nc.vector.tensor_scalar_add(
                            out=dslice, in0=pq[:], scalar1=bqk_q[:, 0:1])
                    elif nch == 1:
                        nc.vector.tensor_scalar_add(
                            out=dslice, in0=pq[:], scalar1=bqk_q[:, 1:2])
                    else:
                        nc.scalar.copy(out=dslice, in_=pq[:])

        def emit_vtrans(b):
            if not warmed[0]:
                emit_warmup()
            # V back-transpose to [token, feat]; single strided evac copy
            # per block into the merged VAB tile (A cols 0:64, B 65+64:).
            for kb in range(NKB):
                pt = psum.tile([P, P], MMDT, tag="py", bufs=4)
                nc.tensor.transpose(
                    pt[:], VT_sb[:, b, kb * P:(kb + 1) * P], idb_sb[:])
                dst = VAB[:, b, kb, :]
                dstv = bass.AP(tensor=dst.tensor, offset=dst.offset,
                               ap=[list(dst.ap[0]), [65 + 64, 2], [1, 64]])
                eng = (nc.vector, nc.scalar)[evac_rr[0] % 2]
                evac_rr[0] += 1
                src = pt.rearrange("p (two f) -> p two f", two=2)
                if eng is nc.scalar:
                    eng.copy(out=dstv, in_=src)
                else:
                    eng.tensor_copy(out=dstv, in_=src)

        use_pb2 = os.environ.get("KERNEL_PB2", "0") == "1"
        norm_fin = []

        def emit_normalize_finish(b, qt, *args):
            yTq = ypool.tile([P, QTW], MMDT, tag="yT", name=f"yT_{b}_{qt}")
            yts[(b, qt)] = yTq
            if use_pb2:
                dsA, dsB = args
                pyA, pyB = pys[(b, qt)]
                rsA = spool.tile([1, QTW], F32, tag="rsA", bufs=2, name=f"rsA_{b}_{qt}")
                rsB = spool.tile([1, QTW], F32, tag="rsB", bufs=2, name=f"rsB_{b}_{qt}")
                nc.vector.reciprocal_approx_fast(out=rsA[:, :], in_=dsA[:, :])
                nc.vector.reciprocal_approx_fast(out=rsB[:, :], in_=dsB[:, :])
                db = spool.tile([P, QTW], F32, tag="db", bufs=2, name=f"db_{b}_{qt}")
                nc.gpsimd.partition_broadcast(db[0:64, :], rsA[0:1, :])
                nc.gpsimd.partition_broadcast(db[64:128, :], rsB[0:1, :])
                nc.vector.tensor_tensor(
                    yTq[0:64, :], pyA[0:64, :], db[0:64, :], mult)
                nc.vector.tensor_tensor(
                    yTq[64:128, :], pyB[64:128, :], db[64:128, :], mult)
            else:
                yu, rb_src = args
                rb = spool.tile([P, QTW], F32, tag="rb", bufs=3, name=f"rb_{b}_{qt}")
                nc.vector.reciprocal_approx_fast(out=rb[:, :], in_=rb_src[:, :])
                nc.vector.tensor_tensor(
                    yTq[0:64, :], yu[0:64, 0:QTW], rb[0:64, :], mult)
                # last qt: keep the gpsimd FIFO clear for the batch's
                # out-DMA issues (a gpsimd mult waiting on the bounce
                # round-trip would head-block them past the kernel end)
                meng = nc.vector if qt == NQT - 1 else nc.gpsimd
                meng.tensor_tensor(
                    yTq[64:128, :], yu[64:128, QTW:2 * QTW], rb[64:128, :], mult)

        def drain_norm(limit=None):
            k = len(norm_fin) if limit is None else min(limit, len(norm_fin))
            for _ in range(k):
                emit_normalize_finish(*norm_fin.pop(0))

        def emit_normalize(b, qt):
            pyA, pyB = pys[(b, qt)]
            if use_pb2:
                dsA = spool.tile([1, QTW], F32, tag="dsA", bufs=2, name=f"dsA_{b}_{qt}")
                dsB = spool.tile([1, QTW], F32, tag="dsB", bufs=2, name=f"dsB_{b}_{qt}")
                nc.scalar.copy(out=dsA[:, :], in_=pyA[64:65, :])
                nc.vector.tensor_copy(out=dsB[:, :], in_=pyB[32:33, :])
                norm_fin.append((b, qt, dsA, dsB))
                return
            if True:
                # fallback: baseline-style DRAM bounce broadcast
                yu = spool.tile([P, 2 * QTW], F32, tag="yu", bufs=4,
                                name=f"yu_{b}_{qt}")
                nc.scalar.copy(out=yu[0:65, 0:QTW], in_=pyA[0:65, :])
                nc.vector.tensor_copy(out=yu[0:128, QTW:2 * QTW], in_=pyB[:, :])
                dr = dpool.tile([2, QTW], F32, tag="dr", bufs=3, name=f"dr_{b}_{qt}")
                nq = nc.gpsimd if b == 0 else nc.sync
                nq.dma_start(out=dr[1:2, :], in_=yu[64:65, 0:QTW])
                nq.dma_start(out=dr[0:1, :], in_=yu[32:33, QTW:2 * QTW])
                dbx = spool.tile([P, QTW], F32, tag="db", bufs=3, name=f"db_{b}_{qt}")
                rowB, rowA = dr[0:1, :], dr[1:2, :]
                srcA = bass.AP(tensor=rowA.tensor, offset=rowA.offset,
                               ap=[[0, 64], [1, QTW]])
                srcB = bass.AP(tensor=rowB.tensor, offset=rowB.offset,
                               ap=[[0, 64], [1, QTW]])
                nq.dma_start(out=dbx[0:64, :], in_=srcA)
                nq.dma_start(out=dbx[64:128, :], in_=srcB)
                norm_fin.append((b, qt, yu, dbx))
                return

        def emit_attnv(b, item):
            qt, kb, e, qoff, first, last = item
            if first:
                pyA = psum.tile([P, QTW], F32, tag="py", bufs=4,
                                name=f"pyA_{b}_{qt}")
                pyB = psum.tile([P, QTW], F32, tag="py", bufs=4,
                                name=f"pyB_{b}_{qt}")
                pys[(b, qt)] = (pyA, pyB)
            pyA, pyB = pys[(b, qt)]
            nc.tensor.matmul(
                pyA[0:65, qoff:QTW], (VAB[:, b, kb, 0:65]),
                (e[:, qoff:QTW]),
                start=first, stop=last, skip_group_check=True,
            )
            nc.tensor.matmul(
                pyB[:, qoff:QTW], (VAB[:, b, kb, 65:65 + P]),
                (e[:, QTW + qoff:2 * QTW]),
                start=first, stop=last, skip_group_check=True,
            )
            if last:
                emit_normalize(b, qt)

        cur_osb = {}

        def emit_proj_sm(b, sm, whole):
            # one token-block of the output projection.  whole=True: po on
            # the (idle) "ps" ring, single [128,1024] evac per sm -- fewer,
            # bigger evacs keep up with back-to-back proj matmuls at the
            # kernel tail.  whole=False: py-ring halves (used interleaved
            # with QKV chains, where evac slack is plentiful).
            yTq = yts[(b, sm // 4)]
            if sm % 4 == 0:
                cur_osb[b] = opool.tile([P, 4, C], BF16, tag="osb",
                                        name=f"osb_{b}_{sm // 4}")
            osb = cur_osb[b][:, sm % 4, :]
            if whole:
                po = psum.tile([P, 2, QTW], F32, tag="ps", bufs=2,
                               name=f"po_{b}_{sm}")
                for nh in range(2):
                    nc.tensor.matmul(
                        po[:, nh, :],
                        (yTq[:, (sm % 4) * P:(sm % 4 + 1) * P]),
                        (wp_sb[:, nh * QTW:(nh + 1) * QTW]),
                        start=True, stop=True,
                    )
                eng = (nc.scalar, nc.vector)[evac_rr[0] % 2]
                evac_rr[0] += 1
                osb3 = osb.rearrange("p (h q) -> p h q", h=2)
                if eng is nc.scalar:
                    eng.copy(out=osb3, in_=po[:, :, :])
                else:
                    eng.tensor_copy(out=osb3, in_=po[:, :, :])
            else:
                for nh in range(2):
                    po = psum.tile([P, QTW], F32, tag="py", bufs=4,
                                   name=f"po_{b}_{sm}_{nh}")
                    nc.tensor.matmul(
                        po[:],
                        (yTq[:, (sm % 4) * P:(sm % 4 + 1) * P]),
                        (wp_sb[:, nh * QTW:(nh + 1) * QTW]),
                        start=True, stop=True,
                    )
                    eng = (nc.scalar, nc.vector)[evac_rr[0] % 2]
                    evac_rr[0] += 1
                    if eng is nc.scalar:
                        eng.copy(out=osb[:, nh * QTW:(nh + 1) * QTW],
                                 in_=po[:])
                    else:
                        eng.tensor_copy(out=osb[:, nh * QTW:(nh + 1) * QTW],
                                        in_=po[:])
            if sm % 4 == 3:
                grp = sm // 4
                dq = nc.sync if b == 0 else nc.gpsimd
                dq.dma_start(out=out4[b * NMT_B + grp], in_=cur_osb[b][:, :, :])

        def emit_attention(b):
            # One flat stream of score-groups, diagonal blocks first within
            # each qt; attnV trails scores by SKEW groups across qt
            # boundaries so the PE never drains waiting on exp.
            SKEW = int(os.environ.get('KERNEL_SKEW', '6'))
            groups = []
            for qt in range(NQT):
                order = list(range(qt * 4, (qt + 1) * 4)) + list(range(0, qt * 4))
                for i, kb in enumerate(order):
                    groups.append((qt, kb, i == 0, i == len(order) - 1))

            pend = []
            for gi, (qt, kb, first, last) in enumerate(groups):
                d = kb - (qt * (QTW // P))
                qoff = 0 if os.environ.get("KERNEL_NARROW", "1") == "0" \
                    else max(0, d) * P
                w = QTW - qoff
                q0 = qt * QTW + qoff
                ps = psum.tile([P, 2 * QTW], F32, tag="ps", bufs=2,
                               name=f"ps_{b}_{qt}_{kb}")
                nc.tensor.matmul(
                    ps[:, qoff:QTW],
                    (KT_sb[0:64, b, kb * P:(kb + 1) * P]),
                    (QT_sb[0:64, b, q0:q0 + w]),
                    start=True, stop=True, tile_position=(0, 0),
                )
                nc.tensor.matmul(
                    ps[:, QTW + qoff:2 * QTW],
                    (KT_sb[64:128, b, kb * P:(kb + 1) * P]),
                    (QT_sb[64:128, b, q0:q0 + w]),
                    start=True, stop=True, tile_position=(64, 0),
                )
                e = epool.tile([P, 2 * QTW], MMDT, tag="e", bufs=8,
                               name=f"e_{b}_{qt}_{kb}")
                ps3 = ps.rearrange("p (h q) -> p h q", h=2)
                e3 = e.rearrange("p (h q) -> p h q", h=2)
                use_dve = (str(gi % 5) in _DVE_PAT) and EXP_OP is not None
                if use_dve:
                    nc.vector._custom_dve(
                        EXP_OP, out=e3[:, :, qoff:], in0=ps3[:, :, qoff:],
                        s0=_EXP_C0, s1=_EXP_C1)
                else:
                    nc.scalar.activation(out=e3[:, :, qoff:],
                                         in_=ps3[:, :, qoff:],
                                         func=Exp, scale=SCALE)
                if d >= 0:
                    # within the window: keep exp[j, h, i'] where i' >= j
                    nc.gpsimd.affine_select(
                        out=e3[:, :, qoff:],
                        in_=e3[:, :, qoff:],
                        pattern=[[0, 2], [1, w]],
                        compare_op=mybir.AluOpType.is_ge,
                        fill=0.0,
                        base=0,
                        channel_multiplier=-1,
                    )
                pend.append((qt, kb, e, qoff, first, last))
                if len(pend) > SKEW:
                    emit_attnv(b, pend.pop(0))
                if gi % 4 == 3:
                    drain_norm(1)
            for item in pend:
                emit_attnv(b, item)
            drain_norm()

        # ---------------- schedule ----------------
        emit_qkv(0)
        emit_vtrans(0)
        emit_attention(0)
        emit_qkv(1, proj_b=0)
        emit_vtrans(1)
        emit_attention(1)
        for sm in range(T // P):
            emit_proj_sm(1, sm, whole=True)

    nc.finalize()
    return nc


def prep_inputs(x, W_qkv, b_qkv, W_proj, b_proj):
    """Host-side sharding: returns list of 8 per-core input dicts."""
    import ml_dtypes
    mmnp = ml_dtypes.bfloat16
    x = np.asarray(x, dtype=np.float32)
    W_qkv = np.asarray(W_qkv, dtype=np.float32)
    b_qkv = np.asarray(b_qkv, dtype=np.float32)
    W_proj = np.asarray(W_proj, dtype=np.float32)

    xT = x.reshape(BT, C).T                                      # [C, BT]
    # tile-matched: [g, p, ko, m] with C = ko*128 + p, token = g*512 + m
    xT = np.ascontiguousarray(
        xT.reshape(KO, P, B * NMT_B, MT).transpose(2, 1, 0, 3)).astype(mmnp)
    ident = np.eye(P, dtype=np.float32)

    in_maps = []
    for c in range(NCORES):
        hA, hB = HPC * c, HPC * c + 1
        cols = []
        for part in range(3):                               # q, k, v
            for h in (hA, hB):
                cols.append(W_qkv[:, part * C + h * D: part * C + (h + 1) * D])
        w = np.concatenate(cols, axis=1)                         # [C, 384]
        w = np.ascontiguousarray(
            w.reshape(KO, P, 3, HPC * D).transpose(1, 2, 0, 3)).astype(mmnp)

        bq = np.concatenate([b_qkv[hA * D:(hA + 1) * D], b_qkv[hB * D:(hB + 1) * D]])
        bk = np.concatenate([b_qkv[C + hA * D: C + (hA + 1) * D],
                             b_qkv[C + hB * D: C + (hB + 1) * D]])
        bqk = np.ascontiguousarray(np.stack([bq, bk], axis=1))  # [128, 2]

        wp = np.ascontiguousarray(W_proj[c * P:(c + 1) * P, :]).astype(mmnp)

        in_maps.append({
            "xT": xT,
            "w": w,
            "bqk": bqk,
            "wp": wp,
            "ident": ident,
        })
    return in_maps


_NC_CACHE = None


def kernel(x, W_qkv, b_qkv, W_proj, b_proj):
    global _NC_CACHE, LAST_RESULT
    from concourse.bass_utils import run_bass_kernel_spmd

    if _NC_CACHE is None:
        _NC_CACHE = build_nc()
    nc = _NC_CACHE

    in_maps = prep_inputs(x, W_qkv, b_qkv, W_proj, b_proj)
    trace = os.environ.get("KERNEL_TRACE", "0") == "1"
    res = run_bass_kernel_spmd(nc, in_maps, list(range(NCORES)), trace=trace)
    LAST_RESULT = res

    acc = np.zeros((BT, C), dtype=np.float64)
    for r in res.results:
        o = r["out"]  # [8, 128, 4, 1024] tile-matched
        acc += o.transpose(0, 2, 1, 3).reshape(BT, C).astype(np.float64)
    # attn rows sum to 1, so the V bias contributes b_v @ W_proj to every
    # token row; add it and the proj bias here (exact, part of unshard).
    W_proj = np.asarray(W_proj, dtype=np.float32)
    b_qkv = np.asarray(b_qkv, dtype=np.float32)
    b_proj = np.asarray(b_proj, dtype=np.float32)
    acc += (b_qkv[2 * C:].astype(np.float64) @ W_proj.astype(np.float64)
            + b_proj.astype(np.float64))
    return acc.astype(np.float32).reshape(B, T, C)
